# revision 8
# baseline (speedup 1.0000x reference)
"""Bahdanau attention kernel for Trainium2, 8 NeuronCores, data-parallel over batch.

Reference computation (per batch b):
    hq = query @ Wq_w.T + Wq_b          # [Q, A]
    hy = y @ Wy_w.T + Wy_b              # [Y, A]
    scores[q, y] = v_w . tanh(hq[q] + hy[y]) + v_b   # [Q, Y]
    att = softmax(scores, axis=y)       # [Q, Y]
    sim[q] = max_y scores[q, y]         # [1, Q]

Shapes: B=8, Q=256, Y=512, D=1024, A=256.

Kernel strategy (one batch per core):
  - Keep the A dim on SBUF partitions (2 tiles of 128).
  - hqT [a, q], hyT [a, y] computed via PE matmuls on transposed operands
    (PE transpose-mode for query/y/W transposes), float32r matmul dtype.
  - Per q: S[a, y] = hyT[a, y] + hqT'[a, q] via DVE tensor_scalar_add
    (per-partition scalar); batches of 16 q's are tanh'd in a single
    big ACT instruction (in-place, bf16).
  - Score dot: PE matmul with a sliding-window stationary ZV (v at
    column 127) so out[q, y] lands directly in a [128, 512] PSUM tile.
  - Softmax: DVE reduce_max -> ACT Exp(bias=-max, accum_out=sum) ->
    DVE reciprocal -> scale.  sim = max + v_b.
"""

import numpy as np

B, Q, Y, D, A = 8, 256, 512, 1024, 256
KT = D // 128   # k tiles in contraction dim
AT = A // 128   # a tiles
QB = Q // 128   # q blocks
YTILES = Y // 128
QTILES = Q // 128
QC = 16         # q's per ACT chunk
NCH = 128 // QC

_cached = None


def _build():
    import concourse.bass as bass
    import concourse.tile as tile
    from concourse import bacc, mybir
    from concourse import masks

    f32 = mybir.dt.float32
    f32r = mybir.dt.float32r
    bf16 = mybir.dt.bfloat16
    ts = bass.ts
    AF = mybir.ActivationFunctionType

    nc = bacc.Bacc("TRN2", target_bir_lowering=False, debug=False)

    query_ext = nc.dram_tensor("query", [Q, D], f32, kind="ExternalInput")
    y_ext = nc.dram_tensor("y", [Y, D], f32, kind="ExternalInput")
    wq_ext = nc.dram_tensor("Wq_w", [A, D], f32, kind="ExternalInput")
    wqb_ext = nc.dram_tensor("Wq_b", [A], f32, kind="ExternalInput")
    wy_ext = nc.dram_tensor("Wy_w", [A, D], f32, kind="ExternalInput")
    wyb_ext = nc.dram_tensor("Wy_b", [A], f32, kind="ExternalInput")
    v_ext = nc.dram_tensor("v_w", [1, A], f32, kind="ExternalInput")
    vb_ext = nc.dram_tensor("v_b", [1], f32, kind="ExternalInput")
    att_ext = nc.dram_tensor("att", [Q, Y], f32, kind="ExternalOutput")
    sim_ext = nc.dram_tensor("sim", [1, Q], f32, kind="ExternalOutput")

    with tile.TileContext(nc) as tc:
        from contextlib import ExitStack
        ctx = ExitStack()
        with ctx:
            consts = ctx.enter_context(tc.tile_pool(name="consts", bufs=1))
            nat_pool = ctx.enter_context(tc.tile_pool(name="nat", bufs=4))
            tr_sb = ctx.enter_context(tc.tile_pool(name="tr_sb", bufs=1))
            s_pool = ctx.enter_context(tc.tile_pool(name="s", bufs=3))
            soft_pool = ctx.enter_context(tc.tile_pool(name="soft", bufs=2))
            small = ctx.enter_context(tc.tile_pool(name="small", bufs=2))
            psum_proj = ctx.enter_context(
                tc.tile_pool(name="ps_proj", bufs=1, space="PSUM"))
            psum_sc = ctx.enter_context(
                tc.tile_pool(name="ps_sc", bufs=2, space="PSUM"))

            # ---- constants ----
            ones_row = consts.tile([1, 128], f32)
            nc.gpsimd.memset(ones_row[:], 1.0)

            # per-partition vectors
            bq_sb = [consts.tile([128, 1], f32, name=f"bq{t}") for t in range(AT)]
            by_sb = [consts.tile([128, 1], f32, name=f"by{t}") for t in range(AT)]
            v_sb = [consts.tile([128, 1], f32, name=f"v{t}") for t in range(AT)]
            vb_sb = consts.tile([1, 1], f32)
            for t in range(AT):
                nc.sync.dma_start(bq_sb[t][:], wqb_ext.ap()[ts(t, 128)].unsqueeze(1))
                nc.sync.dma_start(by_sb[t][:], wyb_ext.ap()[ts(t, 128)].unsqueeze(1))
                nc.sync.dma_start(v_sb[t][:], v_ext.ap()[0, ts(t, 128)].unsqueeze(1))
            nc.sync.dma_start(vb_sb[:], vb_ext.ap().unsqueeze(0))

            # combined bias (Wq_b + Wy_b) per a-tile
            cb = [consts.tile([128, 1], f32, name=f"cb{t}") for t in range(AT)]
            for t in range(AT):
                nc.vector.tensor_add(cb[t][:], bq_sb[t][:], by_sb[t][:])

            # vb broadcast to 128 partitions via K=1 matmul with ones
            ps_vb = psum_proj.tile([128, 1], f32, tag="hy")
            nc.tensor.matmul(ps_vb[:], ones_row[:], vb_sb[:], start=True, stop=True)
            vb_bc = consts.tile([128, 1], f32)
            nc.vector.tensor_copy(vb_bc[:], ps_vb[:])

            # ZV sliding-window stationaries: v at column 127, zeros elsewhere
            zv = [consts.tile([128, 256], bf16, name=f"zv{t}") for t in range(AT)]
            for t in range(AT):
                nc.gpsimd.memset(zv[t][:], 0.0)
                nc.vector.tensor_copy(zv[t][:, 127:128], v_sb[t][:])

            # ---- load + transpose query / y / weights ----
            # transposed f32 buffers: per k-tile
            qT = [tr_sb.tile([128, Q], bf16, name=f"qT{k}") for k in range(KT)]
            yT = [tr_sb.tile([128, Y], bf16, name=f"yT{k}") for k in range(KT)]
            wqT = [tr_sb.tile([128, A], bf16, name=f"wqT{k}") for k in range(KT)]
            wyT = [tr_sb.tile([128, A], bf16, name=f"wyT{k}") for k in range(KT)]

            def load_and_transpose(ext, nrows, dst):
                # ext: [nrows*128, D] DRAM f32.
                # DMA in -> DVE cast to bf16 -> xbar DMA transpose per
                # 128x128 block into dst[k][:, ts(i,128)].
                for i in range(nrows):
                    nat = nat_pool.tile([128, D], f32, tag="nat")
                    nc.sync.dma_start(nat[:], ext.ap()[ts(i, 128), :])
                    natb = nat_pool.tile([128, D], bf16, tag="natb")
                    nc.vector.tensor_copy(natb[:], nat[:])
                    for k in range(KT):
                        nc.sync.dma_start(dst[k][:, ts(i, 128)],
                                          natb[:, ts(k, 128)], transpose=True)

            # y-side first: the main loop's first chunk needs hyT fully
            # and hqT only for its first few q columns.
            load_and_transpose(wy_ext, AT, wyT)
            load_and_transpose(y_ext, YTILES, yT)
            load_and_transpose(wq_ext, AT, wqT)
            load_and_transpose(query_ext, QTILES, qT)

            # ---- projections ----
            # hyT[a, y] (bf16, no bias)
            hyT = [tr_sb.tile([128, Y], bf16, name=f"hyT{t}") for t in range(AT)]
            for t in range(AT):
                ps = psum_proj.tile([128, Y], f32, tag="hy")
                for k in range(KT):
                    nc.tensor.matmul(ps[:], wyT[k][:, ts(t, 128)], yT[k][:],
                                     start=(k == 0), stop=(k == KT - 1))
                nc.vector.tensor_copy(hyT[t][:], ps[:])

            # hqT'[a, q] = sum_d WqT[d, a] * qT[d, q] + (bq + by)   (f32)
            hqT = [tr_sb.tile([128, Q], f32, name=f"hqT{t}") for t in range(AT)]
            for t in range(AT):
                ps = psum_proj.tile([128, Q], f32, tag="hq")
                for k in range(KT):
                    nc.tensor.matmul(ps[:], wqT[k][:, ts(t, 128)], qT[k][:],
                                     start=(k == 0), stop=(k == KT - 1))
                nc.vector.tensor_scalar_add(hqT[t][:], ps[:], cb[t][:, 0:1])

            # ---- main loop: tanh + score dot, then softmax per q-block ----
            for qb in range(QB):
                ps_scores = psum_sc.tile([128, Y], f32, tag="scores")
                for ch in range(NCH):
                    for t in range(AT):
                        S = s_pool.tile([128, QC * Y], bf16, tag="S")
                        for j in range(QC):
                            q = qb * 128 + ch * QC + j
                            nc.vector.tensor_scalar_add(
                                S[:, ts(j, Y)], hyT[t][:], hqT[t][:, q:q + 1])
                        nc.scalar.activation(S[:], S[:], AF.Tanh)
                        for j in range(QC):
                            ql = ch * QC + j
                            first = (ch == 0 and t == 0 and j == 0)
                            last = (ch == NCH - 1 and t == AT - 1 and j == QC - 1)
                            nc.tensor.matmul(
                                ps_scores[:],
                                zv[t][:, 127 - ql:255 - ql],
                                S[:, ts(j, Y)],
                                start=first, stop=last)

                # softmax over y (free dim) for this 128-q block
                mx = small.tile([128, 1], f32, tag="mx")
                nc.vector.reduce_max(mx[:], ps_scores[:], axis=mybir.AxisListType.X)
                nmx = small.tile([128, 1], f32, tag="nmx")
                nc.vector.tensor_scalar_mul(nmx[:], mx[:], -1.0)
                e_sb = soft_pool.tile([128, Y], f32, tag="e")
                sum_e = small.tile([128, 1], f32, tag="sum")
                nc.scalar.activation(e_sb[:], ps_scores[:], AF.Exp,
                                     bias=nmx[:, 0:1], accum_out=sum_e[:, 0:1])
                rinv = small.tile([128, 1], f32, tag="rinv")
                nc.vector.reciprocal(rinv[:], sum_e[:])
                nc.vector.tensor_scalar_mul(e_sb[:], e_sb[:], rinv[:, 0:1])
                nc.sync.dma_start(att_ext.ap()[ts(qb, 128), :], e_sb[:])

                sim_sb = small.tile([128, 1], f32, tag="sim")
                nc.vector.tensor_add(sim_sb[:], mx[:], vb_bc[:])
                nc.sync.dma_start(sim_ext.ap()[0:1, ts(qb, 128)], sim_sb[:])

    nc.compile()
    return nc


def _get_nc():
    global _cached
    if _cached is None:
        _cached = _build()
    return _cached


def kernel(query, y, Wq_w, Wq_b, Wy_w, Wy_b, v_w, v_b):
    from concourse.bass_utils import run_bass_kernel_spmd

    nc = _get_nc()
    in_maps = []
    for b in range(B):
        in_maps.append({
            "query": np.ascontiguousarray(query[b], dtype=np.float32),
            "y": np.ascontiguousarray(y[b], dtype=np.float32),
            "Wq_w": np.ascontiguousarray(Wq_w, dtype=np.float32),
            "Wq_b": np.ascontiguousarray(Wq_b, dtype=np.float32),
            "Wy_w": np.ascontiguousarray(Wy_w, dtype=np.float32),
            "Wy_b": np.ascontiguousarray(Wy_b, dtype=np.float32),
            "v_w": np.ascontiguousarray(v_w, dtype=np.float32),
            "v_b": np.ascontiguousarray(v_b, dtype=np.float32),
        })
    res = run_bass_kernel_spmd(nc, in_maps, core_ids=list(range(B)))
    att = np.stack([res.results[b]["att"] for b in range(B)])
    sim = np.stack([res.results[b]["sim"] for b in range(B)])
    return att.astype(np.float32), sim.astype(np.float32)


# revision 9
# speedup vs baseline: 1.4128x; 1.4128x over previous
"""Bahdanau attention kernel for Trainium2, 8 NeuronCores, data-parallel over batch.

Reference computation (per batch b):
    hq = query @ Wq_w.T + Wq_b          # [Q, A]
    hy = y @ Wy_w.T + Wy_b              # [Y, A]
    scores[q, y] = v_w . tanh(hq[q] + hy[y]) + v_b   # [Q, Y]
    att = softmax(scores, axis=y)       # [Q, Y]
    sim[q] = max_y scores[q, y]         # [1, Q]

Shapes: B=8, Q=256, Y=512, D=1024, A=256.

Kernel strategy (one batch per core):
  - Keep the A dim on SBUF partitions (2 tiles of 128).
  - hqT [a, q], hyT [a, y] computed via PE matmuls on transposed operands
    (PE transpose-mode for query/y/W transposes), float32r matmul dtype.
  - Per q: S[a, y] = hyT[a, y] + hqT'[a, q] via DVE tensor_scalar_add
    (per-partition scalar); batches of 16 q's are tanh'd in a single
    big ACT instruction (in-place, bf16).
  - Score dot: PE matmul with a sliding-window stationary ZV (v at
    column 127) so out[q, y] lands directly in a [128, 512] PSUM tile.
  - Softmax: DVE reduce_max -> ACT Exp(bias=-max, accum_out=sum) ->
    DVE reciprocal -> scale.  sim = max + v_b.
"""

import numpy as np

B, Q, Y, D, A = 8, 256, 512, 1024, 256
KT = D // 128   # k tiles in contraction dim
AT = A // 128   # a tiles
QB = Q // 128   # q blocks
YTILES = Y // 128
QTILES = Q // 128
QC = 16         # q's per ACT chunk
NCH = 128 // QC

_cached = None


def _build():
    import concourse.bass as bass
    import concourse.tile as tile
    from concourse import bacc, mybir
    from concourse import masks

    f32 = mybir.dt.float32
    f32r = mybir.dt.float32r
    bf16 = mybir.dt.bfloat16
    ts = bass.ts
    AF = mybir.ActivationFunctionType

    nc = bacc.Bacc("TRN2", target_bir_lowering=False, debug=False)

    query_ext = nc.dram_tensor("query", [Q, D], f32, kind="ExternalInput")
    y_ext = nc.dram_tensor("y", [Y, D], f32, kind="ExternalInput")
    wq_ext = nc.dram_tensor("Wq_w", [A, D], f32, kind="ExternalInput")
    wqb_ext = nc.dram_tensor("Wq_b", [A], f32, kind="ExternalInput")
    wy_ext = nc.dram_tensor("Wy_w", [A, D], f32, kind="ExternalInput")
    wyb_ext = nc.dram_tensor("Wy_b", [A], f32, kind="ExternalInput")
    v_ext = nc.dram_tensor("v_w", [1, A], f32, kind="ExternalInput")
    vb_ext = nc.dram_tensor("v_b", [1], f32, kind="ExternalInput")
    att_ext = nc.dram_tensor("att", [Q, Y], f32, kind="ExternalOutput")
    sim_ext = nc.dram_tensor("sim", [1, Q], f32, kind="ExternalOutput")

    with tile.TileContext(nc) as tc:
        from contextlib import ExitStack
        ctx = ExitStack()
        with ctx:
            consts = ctx.enter_context(tc.tile_pool(name="consts", bufs=1))
            nat_pool = ctx.enter_context(tc.tile_pool(name="nat", bufs=4))
            tr_sb = ctx.enter_context(tc.tile_pool(name="tr_sb", bufs=1))
            s_pool = ctx.enter_context(tc.tile_pool(name="s", bufs=3))
            soft_pool = ctx.enter_context(tc.tile_pool(name="soft", bufs=2))
            small = ctx.enter_context(tc.tile_pool(name="small", bufs=2))
            psum_tr = ctx.enter_context(
                tc.tile_pool(name="ps_tr", bufs=3, space="PSUM"))
            psum_proj = ctx.enter_context(
                tc.tile_pool(name="ps_proj", bufs=1, space="PSUM"))
            psum_sc = ctx.enter_context(
                tc.tile_pool(name="ps_sc", bufs=2, space="PSUM"))

            # ---- constants ----
            ones_row = consts.tile([1, 128], f32)
            nc.gpsimd.memset(ones_row[:], 1.0)

            # per-partition vectors
            bq_sb = [consts.tile([128, 1], f32, name=f"bq{t}") for t in range(AT)]
            by_sb = [consts.tile([128, 1], f32, name=f"by{t}") for t in range(AT)]
            v_sb = [consts.tile([128, 1], f32, name=f"v{t}") for t in range(AT)]
            vb_sb = consts.tile([1, 1], f32)
            for t in range(AT):
                nc.sync.dma_start(bq_sb[t][:], wqb_ext.ap()[ts(t, 128)].unsqueeze(1))
                nc.sync.dma_start(by_sb[t][:], wyb_ext.ap()[ts(t, 128)].unsqueeze(1))
                nc.sync.dma_start(v_sb[t][:], v_ext.ap()[0, ts(t, 128)].unsqueeze(1))
            nc.sync.dma_start(vb_sb[:], vb_ext.ap().unsqueeze(0))

            # combined bias (Wq_b + Wy_b) per a-tile
            cb = [consts.tile([128, 1], f32, name=f"cb{t}") for t in range(AT)]
            for t in range(AT):
                nc.vector.tensor_add(cb[t][:], bq_sb[t][:], by_sb[t][:])

            # vb broadcast to 128 partitions via K=1 matmul with ones
            ps_vb = psum_proj.tile([128, 1], f32, tag="hy")
            nc.tensor.matmul(ps_vb[:], ones_row[:], vb_sb[:], start=True, stop=True)
            vb_bc = consts.tile([128, 1], f32)
            nc.vector.tensor_copy(vb_bc[:], ps_vb[:])

            # ZV sliding-window stationaries: v at column 127, zeros elsewhere
            zv = [consts.tile([128, 256], bf16, name=f"zv{t}") for t in range(AT)]
            for t in range(AT):
                nc.gpsimd.memset(zv[t][:], 0.0)
                nc.vector.tensor_copy(zv[t][:, 127:128], v_sb[t][:])

            # ---- load + transpose query / y / weights ----
            # bf16 identity for PE transpose-mode
            ident = consts.tile([128, 128], bf16)
            masks.make_identity(nc, ident[:])

            # transposed bf16 buffers: single tensor per input, block k at
            # columns [k*M, (k+1)*M) where M is the input's row count.
            qT = tr_sb.tile([128, KT * Q], bf16, name="qT")
            yT = tr_sb.tile([128, KT * Y], bf16, name="yT")
            wqT = tr_sb.tile([128, KT * A], bf16, name="wqT")
            wyT = tr_sb.tile([128, KT * A], bf16, name="wyT")

            def load_and_transpose(ext, nrows, dst):
                # ext: [nrows*128, D] DRAM f32.  DMA in -> DVE cast bf16 ->
                # 8 PE transposes into one bf16 PSUM bank -> one strided
                # DVE copy into dst (block k at column k*M + i*128).
                M = nrows * 128
                dst3 = dst[:].rearrange("p (k m) -> p k m", k=KT)
                for i in range(nrows):
                    nat = nat_pool.tile([128, D], f32, tag="nat")
                    nc.sync.dma_start(nat[:], ext.ap()[ts(i, 128), :])
                    natb = nat_pool.tile([128, D], bf16, tag="natb")
                    nc.vector.tensor_copy(natb[:], nat[:])
                    ps = psum_tr.tile([128, D], bf16, tag="tr")
                    for k in range(KT):
                        nc.tensor.transpose(ps[:, ts(k, 128)],
                                            natb[:, ts(k, 128)], ident[:])
                    nc.vector.tensor_copy(dst3[:, :, ts(i, 128)], ps[:])

            # y-side first: the main loop's first chunk needs hyT fully
            # and hqT only for its first few q columns.
            load_and_transpose(wy_ext, AT, wyT)
            load_and_transpose(y_ext, YTILES, yT)
            load_and_transpose(wq_ext, AT, wqT)
            load_and_transpose(query_ext, QTILES, qT)

            # ---- projections ----
            # hyT[a, y] (bf16, no bias)
            hyT = [tr_sb.tile([128, Y], bf16, name=f"hyT{t}") for t in range(AT)]
            for t in range(AT):
                ps = psum_proj.tile([128, Y], f32, tag="hy")
                for k in range(KT):
                    nc.tensor.matmul(ps[:], wyT[:, k * A + t * 128:k * A + t * 128 + 128],
                                     yT[:, ts(k, Y)],
                                     start=(k == 0), stop=(k == KT - 1))
                nc.vector.tensor_copy(hyT[t][:], ps[:])

            # hqT'[a, q] = sum_d WqT[d, a] * qT[d, q] + (bq + by)   (f32)
            hqT = [tr_sb.tile([128, Q], f32, name=f"hqT{t}") for t in range(AT)]
            for t in range(AT):
                ps = psum_proj.tile([128, Q], f32, tag="hq")
                for k in range(KT):
                    nc.tensor.matmul(ps[:], wqT[:, k * A + t * 128:k * A + t * 128 + 128],
                                     qT[:, ts(k, Q)],
                                     start=(k == 0), stop=(k == KT - 1))
                nc.vector.tensor_scalar_add(hqT[t][:], ps[:], cb[t][:, 0:1])

            # ---- main loop: tanh + score dot, then softmax per q-block ----
            for qb in range(QB):
                ps_scores = psum_sc.tile([128, Y], f32, tag="scores")
                for ch in range(NCH):
                    for t in range(AT):
                        S = s_pool.tile([128, QC * Y], bf16, tag="S")
                        for j in range(QC):
                            q = qb * 128 + ch * QC + j
                            nc.vector.tensor_scalar_add(
                                S[:, ts(j, Y)], hyT[t][:], hqT[t][:, q:q + 1])
                        nc.scalar.activation(S[:], S[:], AF.Tanh)
                        for j in range(QC):
                            ql = ch * QC + j
                            first = (ch == 0 and t == 0 and j == 0)
                            last = (ch == NCH - 1 and t == AT - 1 and j == QC - 1)
                            nc.tensor.matmul(
                                ps_scores[:],
                                zv[t][:, 127 - ql:255 - ql],
                                S[:, ts(j, Y)],
                                start=first, stop=last)

                # softmax over y (free dim) for this 128-q block
                mx = small.tile([128, 1], f32, tag="mx")
                nc.vector.reduce_max(mx[:], ps_scores[:], axis=mybir.AxisListType.X)
                nmx = small.tile([128, 1], f32, tag="nmx")
                nc.vector.tensor_scalar_mul(nmx[:], mx[:], -1.0)
                e_sb = soft_pool.tile([128, Y], f32, tag="e")
                sum_e = small.tile([128, 1], f32, tag="sum")
                nc.scalar.activation(e_sb[:], ps_scores[:], AF.Exp,
                                     bias=nmx[:, 0:1], accum_out=sum_e[:, 0:1])
                rinv = small.tile([128, 1], f32, tag="rinv")
                nc.vector.reciprocal(rinv[:], sum_e[:])
                nc.vector.tensor_scalar_mul(e_sb[:], e_sb[:], rinv[:, 0:1])
                nc.sync.dma_start(att_ext.ap()[ts(qb, 128), :], e_sb[:])

                sim_sb = small.tile([128, 1], f32, tag="sim")
                nc.vector.tensor_add(sim_sb[:], mx[:], vb_bc[:])
                nc.sync.dma_start(sim_ext.ap()[0:1, ts(qb, 128)], sim_sb[:])

    nc.compile()
    return nc


def _get_nc():
    global _cached
    if _cached is None:
        _cached = _build()
    return _cached


def kernel(query, y, Wq_w, Wq_b, Wy_w, Wy_b, v_w, v_b):
    from concourse.bass_utils import run_bass_kernel_spmd

    nc = _get_nc()
    in_maps = []
    for b in range(B):
        in_maps.append({
            "query": np.ascontiguousarray(query[b], dtype=np.float32),
            "y": np.ascontiguousarray(y[b], dtype=np.float32),
            "Wq_w": np.ascontiguousarray(Wq_w, dtype=np.float32),
            "Wq_b": np.ascontiguousarray(Wq_b, dtype=np.float32),
            "Wy_w": np.ascontiguousarray(Wy_w, dtype=np.float32),
            "Wy_b": np.ascontiguousarray(Wy_b, dtype=np.float32),
            "v_w": np.ascontiguousarray(v_w, dtype=np.float32),
            "v_b": np.ascontiguousarray(v_b, dtype=np.float32),
        })
    res = run_bass_kernel_spmd(nc, in_maps, core_ids=list(range(B)))
    att = np.stack([res.results[b]["att"] for b in range(B)])
    sim = np.stack([res.results[b]["sim"] for b in range(B)])
    return att.astype(np.float32), sim.astype(np.float32)


# revision 10
# speedup vs baseline: 1.4254x; 1.0089x over previous
"""Bahdanau attention kernel for Trainium2, 8 NeuronCores, data-parallel over batch.

Reference computation (per batch b):
    hq = query @ Wq_w.T + Wq_b          # [Q, A]
    hy = y @ Wy_w.T + Wy_b              # [Y, A]
    scores[q, y] = v_w . tanh(hq[q] + hy[y]) + v_b   # [Q, Y]
    att = softmax(scores, axis=y)       # [Q, Y]
    sim[q] = max_y scores[q, y]         # [1, Q]

Shapes: B=8, Q=256, Y=512, D=1024, A=256.

Kernel strategy (one batch per core):
  - Keep the A dim on SBUF partitions (2 tiles of 128).
  - hqT [a, q], hyT [a, y] computed via PE matmuls on transposed operands
    (PE transpose-mode for query/y/W transposes), float32r matmul dtype.
  - Per q: S[a, y] = hyT[a, y] + hqT'[a, q] via DVE tensor_scalar_add
    (per-partition scalar); batches of 16 q's are tanh'd in a single
    big ACT instruction (in-place, bf16).
  - Score dot: PE matmul with a sliding-window stationary ZV (v at
    column 127) so out[q, y] lands directly in a [128, 512] PSUM tile.
  - Softmax: DVE reduce_max -> ACT Exp(bias=-max, accum_out=sum) ->
    DVE reciprocal -> scale.  sim = max + v_b.
"""

import numpy as np

B, Q, Y, D, A = 8, 256, 512, 1024, 256
KT = D // 128   # k tiles in contraction dim
AT = A // 128   # a tiles
QB = Q // 128   # q blocks
YTILES = Y // 128
QTILES = Q // 128
QC = 16         # q's per ACT chunk
NCH = 128 // QC

_cached = None


def _build():
    import concourse.bass as bass
    import concourse.tile as tile
    from concourse import bacc, mybir
    from concourse import masks

    f32 = mybir.dt.float32
    f32r = mybir.dt.float32r
    bf16 = mybir.dt.bfloat16
    ts = bass.ts
    AF = mybir.ActivationFunctionType

    nc = bacc.Bacc("TRN2", target_bir_lowering=False, debug=False)

    query_ext = nc.dram_tensor("query", [Q, D], f32, kind="ExternalInput")
    y_ext = nc.dram_tensor("y", [Y, D], f32, kind="ExternalInput")
    wq_ext = nc.dram_tensor("Wq_w", [A, D], f32, kind="ExternalInput")
    wqb_ext = nc.dram_tensor("Wq_b", [A], f32, kind="ExternalInput")
    wy_ext = nc.dram_tensor("Wy_w", [A, D], f32, kind="ExternalInput")
    wyb_ext = nc.dram_tensor("Wy_b", [A], f32, kind="ExternalInput")
    v_ext = nc.dram_tensor("v_w", [1, A], f32, kind="ExternalInput")
    vb_ext = nc.dram_tensor("v_b", [1], f32, kind="ExternalInput")
    att_ext = nc.dram_tensor("att", [Q, Y], f32, kind="ExternalOutput")
    sim_ext = nc.dram_tensor("sim", [1, Q], f32, kind="ExternalOutput")

    with tile.TileContext(nc) as tc:
        from contextlib import ExitStack
        ctx = ExitStack()
        with ctx:
            consts = ctx.enter_context(tc.tile_pool(name="consts", bufs=1))
            nat_pool = ctx.enter_context(tc.tile_pool(name="nat", bufs=4))
            tr_sb = ctx.enter_context(tc.tile_pool(name="tr_sb", bufs=1))
            s_pool = ctx.enter_context(tc.tile_pool(name="s", bufs=3))
            soft_pool = ctx.enter_context(tc.tile_pool(name="soft", bufs=2))
            small = ctx.enter_context(tc.tile_pool(name="small", bufs=2))
            psum_tr = ctx.enter_context(
                tc.tile_pool(name="ps_tr", bufs=3, space="PSUM"))
            psum_proj = ctx.enter_context(
                tc.tile_pool(name="ps_proj", bufs=1, space="PSUM"))
            psum_sc = ctx.enter_context(
                tc.tile_pool(name="ps_sc", bufs=2, space="PSUM"))

            # ---- constants ----
            ones_row = consts.tile([1, 128], f32)
            nc.gpsimd.memset(ones_row[:], 1.0)

            # per-partition vectors
            bq_sb = [consts.tile([128, 1], f32, name=f"bq{t}") for t in range(AT)]
            by_sb = [consts.tile([128, 1], f32, name=f"by{t}") for t in range(AT)]
            v_sb = [consts.tile([128, 1], f32, name=f"v{t}") for t in range(AT)]
            vb_sb = consts.tile([1, 1], f32)
            for t in range(AT):
                nc.sync.dma_start(bq_sb[t][:], wqb_ext.ap()[ts(t, 128)].unsqueeze(1))
                nc.sync.dma_start(by_sb[t][:], wyb_ext.ap()[ts(t, 128)].unsqueeze(1))
                nc.sync.dma_start(v_sb[t][:], v_ext.ap()[0, ts(t, 128)].unsqueeze(1))
            nc.sync.dma_start(vb_sb[:], vb_ext.ap().unsqueeze(0))

            # combined bias (Wq_b + Wy_b) per a-tile
            cb = [consts.tile([128, 1], f32, name=f"cb{t}") for t in range(AT)]
            for t in range(AT):
                nc.vector.tensor_add(cb[t][:], bq_sb[t][:], by_sb[t][:])

            # vb broadcast to 128 partitions via K=1 matmul with ones
            ps_vb = psum_proj.tile([128, 1], f32, tag="hy")
            nc.tensor.matmul(ps_vb[:], ones_row[:], vb_sb[:], start=True, stop=True)
            vb_bc = consts.tile([128, 1], f32)
            nc.vector.tensor_copy(vb_bc[:], ps_vb[:])

            # ZV sliding-window stationaries: v at column 127, zeros elsewhere
            zv = [consts.tile([128, 256], bf16, name=f"zv{t}") for t in range(AT)]
            for t in range(AT):
                nc.gpsimd.memset(zv[t][:], 0.0)
                nc.vector.tensor_copy(zv[t][:, 127:128], v_sb[t][:])

            # ---- load + transpose query / y / weights ----
            # bf16 identity for PE transpose-mode
            ident = consts.tile([128, 128], bf16)
            masks.make_identity(nc, ident[:])

            # transposed bf16 buffers: single tensor per input, block k at
            # columns [k*M, (k+1)*M) where M is the input's row count.
            qT = tr_sb.tile([128, KT * Q], bf16, name="qT")
            yT = tr_sb.tile([128, KT * Y], bf16, name="yT")
            wqT = tr_sb.tile([128, KT * A], bf16, name="wqT")
            wyT = tr_sb.tile([128, KT * A], bf16, name="wyT")

            def load_tile(ext, i):
                # DMA one [128, D] f32 row-tile in and cast to bf16.
                nat = nat_pool.tile([128, D], f32, tag="nat")
                nc.sync.dma_start(nat[:], ext.ap()[ts(i, 128), :])
                natb = nat_pool.tile([128, D], bf16, tag="natb")
                nc.vector.tensor_copy(natb[:], nat[:])
                return natb

            def transpose_tile(natb, nrows, dst, i):
                # 8 PE transposes of natb's 128x128 blocks into one bf16
                # PSUM bank, then one strided DVE copy into dst
                # (block k at column k*nrows*128 + i*128).
                dst3 = dst[:].rearrange("p (k m) -> p k m", k=KT)
                ps = psum_tr.tile([128, D], bf16, tag="tr")
                for k in range(KT):
                    nc.tensor.transpose(ps[:, ts(k, 128)],
                                        natb[:, ts(k, 128)], ident[:])
                nc.vector.tensor_copy(dst3[:, :, ts(i, 128)], ps[:])

            def lat(ext, nrows, dst, only_i=None):
                for i in range(nrows) if only_i is None else [only_i]:
                    transpose_tile(load_tile(ext, i), nrows, dst, i)

            hyT = [tr_sb.tile([128, Y], bf16, name=f"hyT{t}") for t in range(AT)]
            hqT = [tr_sb.tile([128, Q], f32, name=f"hqT{t}") for t in range(AT)]

            def proj_hy(t):
                ps = psum_proj.tile([128, Y], f32, tag="hy")
                for k in range(KT):
                    nc.tensor.matmul(
                        ps[:], wyT[:, k * A + t * 128:k * A + t * 128 + 128],
                        yT[:, ts(k, Y)],
                        start=(k == 0), stop=(k == KT - 1))
                nc.vector.tensor_copy(hyT[t][:], ps[:])

            def proj_hq(t):
                # split into q-halves: half h is ready as soon as query
                # row-tile h is transposed; separate accumulation groups.
                ps = psum_proj.tile([128, Q], f32, tag="hq")
                for h in range(QTILES):
                    for k in range(KT):
                        nc.tensor.matmul(
                            ps[:, ts(h, 128)],
                            wqT[:, k * A + t * 128:k * A + t * 128 + 128],
                            qT[:, k * Q + h * 128:k * Q + h * 128 + 128],
                            start=(k == 0), stop=(k == KT - 1))
                    nc.vector.tensor_scalar_add(
                        hqT[t][:, ts(h, 128)], ps[:, ts(h, 128)], cb[t][:, 0:1])

            # Emission order drives the schedule: minimal t=0 path first so
            # the ACT main loop starts ASAP; the whole t=1 side and the
            # second query half hide under the first ~50us of t=0 tanh.
            lat(wy_ext, AT, wyT, only_i=0)
            lat(y_ext, YTILES, yT)
            proj_hy(0)
            lat(wq_ext, AT, wqT, only_i=0)
            lat(query_ext, QTILES, qT)
            proj_hq(0)
            lat(wy_ext, AT, wyT, only_i=1)
            proj_hy(1)
            lat(wq_ext, AT, wqT, only_i=1)
            proj_hq(1)

            # ---- main loop: tanh + score dot, then softmax per q-block ----
            # t-major within each q-block: t=1 inputs are only needed after
            # ~56us of t=0 work.
            for qb in range(QB):
                ps_scores = psum_sc.tile([128, Y], f32, tag="scores")
                for t in range(AT):
                    for ch in range(NCH):
                        S = s_pool.tile([128, QC * Y], bf16, tag="S")
                        for j in range(QC):
                            q = qb * 128 + ch * QC + j
                            nc.vector.tensor_scalar_add(
                                S[:, ts(j, Y)], hyT[t][:], hqT[t][:, q:q + 1])
                        nc.scalar.activation(S[:], S[:], AF.Tanh)
                        for j in range(QC):
                            ql = ch * QC + j
                            first = (ch == 0 and t == 0 and j == 0)
                            last = (ch == NCH - 1 and t == AT - 1 and j == QC - 1)
                            nc.tensor.matmul(
                                ps_scores[:],
                                zv[t][:, 127 - ql:255 - ql],
                                S[:, ts(j, Y)],
                                start=first, stop=last)

                # softmax over y (free dim) for this 128-q block
                mx = small.tile([128, 1], f32, tag="mx")
                nc.vector.reduce_max(mx[:], ps_scores[:], axis=mybir.AxisListType.X)
                nmx = small.tile([128, 1], f32, tag="nmx")
                nc.vector.tensor_scalar_mul(nmx[:], mx[:], -1.0)
                e_sb = soft_pool.tile([128, Y], f32, tag="e")
                sum_e = small.tile([128, 1], f32, tag="sum")
                nc.scalar.activation(e_sb[:], ps_scores[:], AF.Exp,
                                     bias=nmx[:, 0:1], accum_out=sum_e[:, 0:1])
                rinv = small.tile([128, 1], f32, tag="rinv")
                nc.vector.reciprocal(rinv[:], sum_e[:])
                nc.vector.tensor_scalar_mul(e_sb[:], e_sb[:], rinv[:, 0:1])
                nc.sync.dma_start(att_ext.ap()[ts(qb, 128), :], e_sb[:])

                sim_sb = small.tile([128, 1], f32, tag="sim")
                nc.vector.tensor_add(sim_sb[:], mx[:], vb_bc[:])
                nc.sync.dma_start(sim_ext.ap()[0:1, ts(qb, 128)], sim_sb[:])

    nc.compile()
    return nc


def _get_nc():
    global _cached
    if _cached is None:
        _cached = _build()
    return _cached


def kernel(query, y, Wq_w, Wq_b, Wy_w, Wy_b, v_w, v_b):
    from concourse.bass_utils import run_bass_kernel_spmd

    nc = _get_nc()
    in_maps = []
    for b in range(B):
        in_maps.append({
            "query": np.ascontiguousarray(query[b], dtype=np.float32),
            "y": np.ascontiguousarray(y[b], dtype=np.float32),
            "Wq_w": np.ascontiguousarray(Wq_w, dtype=np.float32),
            "Wq_b": np.ascontiguousarray(Wq_b, dtype=np.float32),
            "Wy_w": np.ascontiguousarray(Wy_w, dtype=np.float32),
            "Wy_b": np.ascontiguousarray(Wy_b, dtype=np.float32),
            "v_w": np.ascontiguousarray(v_w, dtype=np.float32),
            "v_b": np.ascontiguousarray(v_b, dtype=np.float32),
        })
    res = run_bass_kernel_spmd(nc, in_maps, core_ids=list(range(B)))
    att = np.stack([res.results[b]["att"] for b in range(B)])
    sim = np.stack([res.results[b]["sim"] for b in range(B)])
    return att.astype(np.float32), sim.astype(np.float32)


# revision 11
# speedup vs baseline: 1.4648x; 1.0276x over previous
"""Bahdanau attention kernel for Trainium2, 8 NeuronCores, data-parallel over batch.

Reference computation (per batch b):
    hq = query @ Wq_w.T + Wq_b          # [Q, A]
    hy = y @ Wy_w.T + Wy_b              # [Y, A]
    scores[q, y] = v_w . tanh(hq[q] + hy[y]) + v_b   # [Q, Y]
    att = softmax(scores, axis=y)       # [Q, Y]
    sim[q] = max_y scores[q, y]         # [1, Q]

Shapes: B=8, Q=256, Y=512, D=1024, A=256.

Kernel strategy (one batch per core):
  - Keep the A dim on SBUF partitions (2 tiles of 128).
  - hqT [a, q], hyT [a, y] computed via PE matmuls on transposed operands
    (PE transpose-mode for query/y/W transposes), float32r matmul dtype.
  - Per q: S[a, y] = hyT[a, y] + hqT'[a, q] via DVE tensor_scalar_add
    (per-partition scalar); batches of 16 q's are tanh'd in a single
    big ACT instruction (in-place, bf16).
  - Score dot: PE matmul with a sliding-window stationary ZV (v at
    column 127) so out[q, y] lands directly in a [128, 512] PSUM tile.
  - Softmax: DVE reduce_max -> ACT Exp(bias=-max, accum_out=sum) ->
    DVE reciprocal -> scale.  sim = max + v_b.
"""

import numpy as np

B, Q, Y, D, A = 8, 256, 512, 1024, 256
KT = D // 128   # k tiles in contraction dim
AT = A // 128   # a tiles
QB = Q // 128   # q blocks
YTILES = Y // 128
QTILES = Q // 128
QC = 16         # q's per ACT chunk
NCH = 128 // QC

_cached = None


def _build():
    import concourse.bass as bass
    import concourse.tile as tile
    from concourse import bacc, mybir
    from concourse import masks

    f32 = mybir.dt.float32
    f32r = mybir.dt.float32r
    bf16 = mybir.dt.bfloat16
    ts = bass.ts
    AF = mybir.ActivationFunctionType

    nc = bacc.Bacc("TRN2", target_bir_lowering=False, debug=False)

    query_ext = nc.dram_tensor("query", [Q, D], f32, kind="ExternalInput")
    y_ext = nc.dram_tensor("y", [Y, D], f32, kind="ExternalInput")
    wq_ext = nc.dram_tensor("Wq_w", [A, D], f32, kind="ExternalInput")
    wqb_ext = nc.dram_tensor("Wq_b", [A], f32, kind="ExternalInput")
    wy_ext = nc.dram_tensor("Wy_w", [A, D], f32, kind="ExternalInput")
    wyb_ext = nc.dram_tensor("Wy_b", [A], f32, kind="ExternalInput")
    v_ext = nc.dram_tensor("v_w", [1, A], f32, kind="ExternalInput")
    vb_ext = nc.dram_tensor("v_b", [1], f32, kind="ExternalInput")
    att_ext = nc.dram_tensor("att", [Q, Y], f32, kind="ExternalOutput")
    sim_ext = nc.dram_tensor("sim", [1, Q], f32, kind="ExternalOutput")

    with tile.TileContext(nc) as tc:
        from contextlib import ExitStack
        ctx = ExitStack()
        with ctx:
            consts = ctx.enter_context(tc.tile_pool(name="consts", bufs=1))
            nat_pool = ctx.enter_context(tc.tile_pool(name="nat", bufs=4))
            tr_sb = ctx.enter_context(tc.tile_pool(name="tr_sb", bufs=1))
            s_pool = ctx.enter_context(tc.tile_pool(name="s", bufs=3))
            soft_pool = ctx.enter_context(tc.tile_pool(name="soft", bufs=2))
            small = ctx.enter_context(tc.tile_pool(name="small", bufs=2))
            psum_tr = ctx.enter_context(
                tc.tile_pool(name="ps_tr", bufs=3, space="PSUM"))
            psum_proj = ctx.enter_context(
                tc.tile_pool(name="ps_proj", bufs=1, space="PSUM"))
            psum_sc = ctx.enter_context(
                tc.tile_pool(name="ps_sc", bufs=2, space="PSUM"))

            # ---- constants ----
            ones_row = consts.tile([1, 128], f32)
            nc.gpsimd.memset(ones_row[:], 1.0)

            # per-partition vectors
            bq_sb = [consts.tile([128, 1], f32, name=f"bq{t}") for t in range(AT)]
            by_sb = [consts.tile([128, 1], f32, name=f"by{t}") for t in range(AT)]
            v_sb = [consts.tile([128, 1], f32, name=f"v{t}") for t in range(AT)]
            vb_sb = consts.tile([1, 1], f32)
            for t in range(AT):
                nc.sync.dma_start(bq_sb[t][:], wqb_ext.ap()[ts(t, 128)].unsqueeze(1))
                nc.sync.dma_start(by_sb[t][:], wyb_ext.ap()[ts(t, 128)].unsqueeze(1))
                nc.sync.dma_start(v_sb[t][:], v_ext.ap()[0, ts(t, 128)].unsqueeze(1))
            nc.sync.dma_start(vb_sb[:], vb_ext.ap().unsqueeze(0))

            # combined bias (Wq_b + Wy_b) per a-tile
            cb = [consts.tile([128, 1], f32, name=f"cb{t}") for t in range(AT)]
            for t in range(AT):
                nc.vector.tensor_add(cb[t][:], bq_sb[t][:], by_sb[t][:])

            # vb broadcast to 128 partitions via K=1 matmul with ones
            ps_vb = psum_proj.tile([128, 1], f32, tag="hy")
            nc.tensor.matmul(ps_vb[:], ones_row[:], vb_sb[:], start=True, stop=True)
            vb_bc = consts.tile([128, 1], f32)
            nc.vector.tensor_copy(vb_bc[:], ps_vb[:])

            # ZV sliding-window stationaries: v at column 127, zeros elsewhere
            zv = [consts.tile([128, 256], bf16, name=f"zv{t}") for t in range(AT)]
            for t in range(AT):
                nc.gpsimd.memset(zv[t][:], 0.0)
                nc.vector.tensor_copy(zv[t][:, 127:128], v_sb[t][:])

            # ---- load + transpose query / y / weights ----
            # bf16 identity for PE transpose-mode
            ident = consts.tile([128, 128], bf16)
            masks.make_identity(nc, ident[:])

            # transposed bf16 buffers: single tensor per input, block k at
            # columns [k*M, (k+1)*M) where M is the input's row count.
            qT = tr_sb.tile([128, KT * Q], bf16, name="qT")
            yT = tr_sb.tile([128, KT * Y], bf16, name="yT")
            wqT = tr_sb.tile([128, KT * A], bf16, name="wqT")
            wyT = tr_sb.tile([128, KT * A], bf16, name="wyT")

            def load_tile(ext, i):
                # DMA one [128, D] f32 row-tile in and cast to bf16.
                nat = nat_pool.tile([128, D], f32, tag="nat")
                nc.sync.dma_start(nat[:], ext.ap()[ts(i, 128), :])
                natb = nat_pool.tile([128, D], bf16, tag="natb")
                nc.vector.tensor_copy(natb[:], nat[:])
                return natb

            def transpose_tile(natb, nrows, dst, i):
                # 8 PE transposes of natb's 128x128 blocks into one bf16
                # PSUM bank, then one strided DVE copy into dst
                # (block k at column k*nrows*128 + i*128).
                dst3 = dst[:].rearrange("p (k m) -> p k m", k=KT)
                ps = psum_tr.tile([128, D], bf16, tag="tr")
                for k in range(KT):
                    nc.tensor.transpose(ps[:, ts(k, 128)],
                                        natb[:, ts(k, 128)], ident[:])
                nc.vector.tensor_copy(dst3[:, :, ts(i, 128)], ps[:])

            def lat(ext, nrows, dst, only_i=None):
                for i in range(nrows) if only_i is None else [only_i]:
                    transpose_tile(load_tile(ext, i), nrows, dst, i)

            hyT = [tr_sb.tile([128, Y], bf16, name=f"hyT{t}") for t in range(AT)]
            hqT = [tr_sb.tile([128, Q], f32, name=f"hqT{t}") for t in range(AT)]

            def proj_hy(t):
                ps = psum_proj.tile([128, Y], f32, tag="hy")
                for k in range(KT):
                    nc.tensor.matmul(
                        ps[:], wyT[:, k * A + t * 128:k * A + t * 128 + 128],
                        yT[:, ts(k, Y)],
                        start=(k == 0), stop=(k == KT - 1))
                nc.vector.tensor_copy(hyT[t][:], ps[:])

            def proj_hq(t):
                # split into q-halves: half h is ready as soon as query
                # row-tile h is transposed; separate accumulation groups.
                ps = psum_proj.tile([128, Q], f32, tag="hq")
                for h in range(QTILES):
                    for k in range(KT):
                        nc.tensor.matmul(
                            ps[:, ts(h, 128)],
                            wqT[:, k * A + t * 128:k * A + t * 128 + 128],
                            qT[:, k * Q + h * 128:k * Q + h * 128 + 128],
                            start=(k == 0), stop=(k == KT - 1))
                    nc.vector.tensor_scalar_add(
                        hqT[t][:, ts(h, 128)], ps[:, ts(h, 128)], cb[t][:, 0:1])

            # ---- main loop pieces ----
            def sweep(qb, t, ps_scores, chunks, first_sweep, last_sweep):
                # one (q-block, a-tile) pass: S = hy + hq[q] (DVE), tanh
                # (ACT, in-place), score-dot MMs into ps_scores.
                q0 = 0
                for ci, qc in enumerate(chunks):
                    S = s_pool.tile([128, QC * Y], bf16, tag="S")
                    for j in range(qc):
                        q = qb * 128 + q0 + j
                        nc.vector.tensor_scalar_add(
                            S[:, ts(j, Y)], hyT[t][:], hqT[t][:, q:q + 1])
                    nc.scalar.activation(S[:, 0:qc * Y], S[:, 0:qc * Y], AF.Tanh)
                    for j in range(qc):
                        ql = q0 + j
                        first = (first_sweep and ci == 0 and j == 0)
                        last = (last_sweep and ci == len(chunks) - 1
                                and j == qc - 1)
                        nc.tensor.matmul(
                            ps_scores[:],
                            zv[t][:, 127 - ql:255 - ql],
                            S[:, ts(j, Y)],
                            start=first, stop=last)
                    q0 += qc

            def softmax_block(qb, ps_scores, split_out):
                mx = small.tile([128, 1], f32, tag="mx")
                nc.vector.reduce_max(mx[:], ps_scores[:], axis=mybir.AxisListType.X)
                sim_sb = small.tile([128, 1], f32, tag="sim")
                nc.vector.tensor_add(sim_sb[:], mx[:], vb_bc[:])
                nc.sync.dma_start(sim_ext.ap()[0:1, ts(qb, 128)], sim_sb[:])
                nmx = small.tile([128, 1], f32, tag="nmx")
                nc.vector.tensor_scalar_mul(nmx[:], mx[:], -1.0)
                e_sb = soft_pool.tile([128, Y], f32, tag="e")
                sum_e = small.tile([128, 1], f32, tag="sum")
                nc.scalar.activation(e_sb[:], ps_scores[:], AF.Exp,
                                     bias=nmx[:, 0:1], accum_out=sum_e[:, 0:1])
                rinv = small.tile([128, 1], f32, tag="rinv")
                nc.vector.reciprocal(rinv[:], sum_e[:])
                if split_out:
                    for h in range(2):
                        nc.vector.tensor_scalar_mul(
                            e_sb[:, ts(h, Y // 2)], e_sb[:, ts(h, Y // 2)],
                            rinv[:, 0:1])
                        nc.sync.dma_start(
                            att_ext.ap()[ts(qb, 128), ts(h, Y // 2)],
                            e_sb[:, ts(h, Y // 2)])
                else:
                    nc.vector.tensor_scalar_mul(e_sb[:], e_sb[:], rinv[:, 0:1])
                    nc.sync.dma_start(att_ext.ap()[ts(qb, 128), :], e_sb[:])

            FULL = [QC] * NCH                       # 8 x 16
            TAIL = [QC] * (NCH - 1) + [8, 4, 4]     # smaller final chunks

            # Emission order drives the schedule: minimal t=0 path first so
            # the ACT main loop starts ASAP; the whole t=1 side hides under
            # the first (qb=0, t=0) tanh sweep (~56us of ACT work).
            lat(wy_ext, AT, wyT, only_i=0)
            lat(y_ext, YTILES, yT)
            proj_hy(0)
            lat(wq_ext, AT, wqT, only_i=0)
            lat(query_ext, QTILES, qT)
            proj_hq(0)

            scores0 = psum_sc.tile([128, Y], f32, tag="scores", name="scores0")
            sweep(0, 0, scores0, FULL, first_sweep=True, last_sweep=False)

            # t=1 prologue now: fills engine idle slots under the sweep above
            lat(wy_ext, AT, wyT, only_i=1)
            proj_hy(1)
            lat(wq_ext, AT, wqT, only_i=1)
            proj_hq(1)

            sweep(0, 1, scores0, FULL, first_sweep=False, last_sweep=True)
            softmax_block(0, scores0, split_out=False)

            scores1 = psum_sc.tile([128, Y], f32, tag="scores", name="scores1")
            sweep(1, 0, scores1, FULL, first_sweep=True, last_sweep=False)
            sweep(1, 1, scores1, TAIL, first_sweep=False, last_sweep=True)
            softmax_block(1, scores1, split_out=True)

    nc.compile()
    return nc


def _get_nc():
    global _cached
    if _cached is None:
        _cached = _build()
    return _cached


def kernel(query, y, Wq_w, Wq_b, Wy_w, Wy_b, v_w, v_b):
    from concourse.bass_utils import run_bass_kernel_spmd

    nc = _get_nc()
    in_maps = []
    for b in range(B):
        in_maps.append({
            "query": np.ascontiguousarray(query[b], dtype=np.float32),
            "y": np.ascontiguousarray(y[b], dtype=np.float32),
            "Wq_w": np.ascontiguousarray(Wq_w, dtype=np.float32),
            "Wq_b": np.ascontiguousarray(Wq_b, dtype=np.float32),
            "Wy_w": np.ascontiguousarray(Wy_w, dtype=np.float32),
            "Wy_b": np.ascontiguousarray(Wy_b, dtype=np.float32),
            "v_w": np.ascontiguousarray(v_w, dtype=np.float32),
            "v_b": np.ascontiguousarray(v_b, dtype=np.float32),
        })
    res = run_bass_kernel_spmd(nc, in_maps, core_ids=list(range(B)))
    att = np.stack([res.results[b]["att"] for b in range(B)])
    sim = np.stack([res.results[b]["sim"] for b in range(B)])
    return att.astype(np.float32), sim.astype(np.float32)


# revision 12
# speedup vs baseline: 1.5313x; 1.0454x over previous
"""Bahdanau attention kernel for Trainium2, 8 NeuronCores, data-parallel over batch.

Reference computation (per batch b):
    hq = query @ Wq_w.T + Wq_b          # [Q, A]
    hy = y @ Wy_w.T + Wy_b              # [Y, A]
    scores[q, y] = v_w . tanh(hq[q] + hy[y]) + v_b   # [Q, Y]
    att = softmax(scores, axis=y)       # [Q, Y]
    sim[q] = max_y scores[q, y]         # [1, Q]

Shapes: B=8, Q=256, Y=512, D=1024, A=256.

Kernel strategy (one batch per core):
  - Keep the A dim on SBUF partitions (2 tiles of 128).
  - hqT [a, q], hyT [a, y] computed via PE matmuls on transposed operands
    (PE transpose-mode for query/y/W transposes), float32r matmul dtype.
  - Per q: S[a, y] = hyT[a, y] + hqT'[a, q] via DVE tensor_scalar_add
    (per-partition scalar); batches of 16 q's are tanh'd in a single
    big ACT instruction (in-place, bf16).
  - Score dot: PE matmul with a sliding-window stationary ZV (v at
    column 127) so out[q, y] lands directly in a [128, 512] PSUM tile.
  - Softmax: DVE reduce_max -> ACT Exp(bias=-max, accum_out=sum) ->
    DVE reciprocal -> scale.  sim = max + v_b.
"""

import numpy as np

B, Q, Y, D, A = 8, 256, 512, 1024, 256
KT = D // 128   # k tiles in contraction dim
AT = A // 128   # a tiles
QB = Q // 128   # q blocks
YTILES = Y // 128
QTILES = Q // 128
QC = 16         # q's per ACT chunk
NCH = 128 // QC

_cached = None


def _build():
    import concourse.bass as bass
    import concourse.tile as tile
    from concourse import bacc, mybir
    from concourse import masks

    f32 = mybir.dt.float32
    f32r = mybir.dt.float32r
    bf16 = mybir.dt.bfloat16
    ts = bass.ts
    AF = mybir.ActivationFunctionType

    nc = bacc.Bacc("TRN2", target_bir_lowering=False, debug=False)

    query_ext = nc.dram_tensor("query", [Q, D], f32, kind="ExternalInput")
    y_ext = nc.dram_tensor("y", [Y, D], f32, kind="ExternalInput")
    wq_ext = nc.dram_tensor("Wq_w", [A, D], f32, kind="ExternalInput")
    wqb_ext = nc.dram_tensor("Wq_b", [A], f32, kind="ExternalInput")
    wy_ext = nc.dram_tensor("Wy_w", [A, D], f32, kind="ExternalInput")
    wyb_ext = nc.dram_tensor("Wy_b", [A], f32, kind="ExternalInput")
    v_ext = nc.dram_tensor("v_w", [1, A], f32, kind="ExternalInput")
    vb_ext = nc.dram_tensor("v_b", [1], f32, kind="ExternalInput")
    att_ext = nc.dram_tensor("att", [Q, Y], f32, kind="ExternalOutput")
    sim_ext = nc.dram_tensor("sim", [1, Q], f32, kind="ExternalOutput")

    with tile.TileContext(nc) as tc:
        from contextlib import ExitStack
        ctx = ExitStack()
        with ctx:
            consts = ctx.enter_context(tc.tile_pool(name="consts", bufs=1))
            nat_pool = ctx.enter_context(tc.tile_pool(name="nat", bufs=4))
            tr_sb = ctx.enter_context(tc.tile_pool(name="tr_sb", bufs=1))
            s_pool = ctx.enter_context(tc.tile_pool(name="s", bufs=3))
            soft_pool = ctx.enter_context(tc.tile_pool(name="soft", bufs=2))
            small = ctx.enter_context(tc.tile_pool(name="small", bufs=2))
            psum_tr = ctx.enter_context(
                tc.tile_pool(name="ps_tr", bufs=3, space="PSUM"))
            psum_proj = ctx.enter_context(
                tc.tile_pool(name="ps_proj", bufs=1, space="PSUM"))
            psum_sc = ctx.enter_context(
                tc.tile_pool(name="ps_sc", bufs=2, space="PSUM"))

            # ---- constants ----
            ones_row = consts.tile([1, 128], f32)
            nc.gpsimd.memset(ones_row[:], 1.0)

            # per-partition vectors (DMAs issued later, after the big
            # input loads, so they don't delay them on the sync queue)
            bq_sb = [consts.tile([128, 1], f32, name=f"bq{t}") for t in range(AT)]
            by_sb = [consts.tile([128, 1], f32, name=f"by{t}") for t in range(AT)]
            v_sb = [consts.tile([128, 1], f32, name=f"v{t}") for t in range(AT)]
            vb_sb = consts.tile([1, 1], f32)
            cb = [consts.tile([128, 1], f32, name=f"cb{t}") for t in range(AT)]
            vb_bc = consts.tile([128, 1], f32)
            zv = [consts.tile([128, 256], bf16, name=f"zv{t}") for t in range(AT)]

            def small_consts():
                for t in range(AT):
                    nc.sync.dma_start(bq_sb[t][:], wqb_ext.ap()[ts(t, 128)].unsqueeze(1))
                    nc.sync.dma_start(by_sb[t][:], wyb_ext.ap()[ts(t, 128)].unsqueeze(1))
                    nc.sync.dma_start(v_sb[t][:], v_ext.ap()[0, ts(t, 128)].unsqueeze(1))
                nc.sync.dma_start(vb_sb[:], vb_ext.ap().unsqueeze(0))
                for t in range(AT):
                    nc.vector.tensor_add(cb[t][:], bq_sb[t][:], by_sb[t][:])
                ps_vb = psum_proj.tile([128, 1], f32, tag="hy")
                nc.tensor.matmul(ps_vb[:], ones_row[:], vb_sb[:], start=True, stop=True)
                nc.vector.tensor_copy(vb_bc[:], ps_vb[:])
                for t in range(AT):
                    nc.gpsimd.memset(zv[t][:], 0.0)
                    nc.vector.tensor_copy(zv[t][:, 127:128], v_sb[t][:])

            # ---- load + transpose query / y / weights ----
            # identities for PE transpose-mode
            ident = consts.tile([128, 128], bf16)
            masks.make_identity(nc, ident[:])
            ident_f32 = consts.tile([128, 128], f32)
            masks.make_identity(nc, ident_f32[:])

            # transposed bf16 buffers: single tensor per input, block k at
            # columns [k*M, (k+1)*M) where M is the input's row count.
            qT = tr_sb.tile([128, KT * Q], bf16, name="qT")
            yT = tr_sb.tile([128, KT * Y], bf16, name="yT")
            wqT = tr_sb.tile([128, KT * A], bf16, name="wqT")
            wyT = tr_sb.tile([128, KT * A], bf16, name="wyT")

            def load_tile(ext, i):
                # DMA one [128, D] f32 row-tile in and cast to bf16.
                nat = nat_pool.tile([128, D], f32, tag="nat")
                nc.sync.dma_start(nat[:], ext.ap()[ts(i, 128), :])
                natb = nat_pool.tile([128, D], bf16, tag="natb")
                nc.vector.tensor_copy(natb[:], nat[:])
                return natb

            def transpose_tile(natb, nrows, dst, i):
                # 8 PE transposes of natb's 128x128 blocks into one bf16
                # PSUM bank, then one strided DVE copy into dst
                # (block k at column k*nrows*128 + i*128).
                dst3 = dst[:].rearrange("p (k m) -> p k m", k=KT)
                ps = psum_tr.tile([128, D], bf16, tag="tr")
                for k in range(KT):
                    nc.tensor.transpose(ps[:, ts(k, 128)],
                                        natb[:, ts(k, 128)], ident[:])
                nc.vector.tensor_copy(dst3[:, :, ts(i, 128)], ps[:])

            def lat(ext, nrows, dst, only_i=None):
                for i in range(nrows) if only_i is None else [only_i]:
                    transpose_tile(load_tile(ext, i), nrows, dst, i)

            hyT = [tr_sb.tile([128, Y], bf16, name=f"hyT{t}") for t in range(AT)]
            hqT = [tr_sb.tile([128, Q], f32, name=f"hqT{t}") for t in range(AT)]

            def proj_hy(t):
                ps = psum_proj.tile([128, Y], f32, tag="hy")
                for k in range(KT):
                    nc.tensor.matmul(
                        ps[:], wyT[:, k * A + t * 128:k * A + t * 128 + 128],
                        yT[:, ts(k, Y)],
                        start=(k == 0), stop=(k == KT - 1))
                nc.vector.tensor_copy(hyT[t][:], ps[:])

            def proj_hq(t, h):
                # one q-half: ready as soon as query row-tile h is transposed
                ps = psum_proj.tile([128, 128], f32, tag="hq")
                for k in range(KT):
                    nc.tensor.matmul(
                        ps[:],
                        wqT[:, k * A + t * 128:k * A + t * 128 + 128],
                        qT[:, k * Q + h * 128:k * Q + h * 128 + 128],
                        start=(k == 0), stop=(k == KT - 1))
                nc.vector.tensor_scalar_add(
                    hqT[t][:, ts(h, 128)], ps[:], cb[t][:, 0:1])

            # ---- main loop pieces ----
            def sweep(qb, t, ps_scores, chunks, first_sweep, last_sweep):
                # one (q-block, a-tile) pass: S = hy + hq[q] (DVE), tanh
                # (ACT, in-place), score-dot MMs into ps_scores.
                q0 = 0
                for ci, qc in enumerate(chunks):
                    S = s_pool.tile([128, QC * Y], bf16, tag="S")
                    for j in range(qc):
                        q = qb * 128 + q0 + j
                        nc.vector.tensor_scalar_add(
                            S[:, ts(j, Y)], hyT[t][:], hqT[t][:, q:q + 1])
                    nc.scalar.activation(S[:, 0:qc * Y], S[:, 0:qc * Y], AF.Tanh)
                    for j in range(qc):
                        ql = q0 + j
                        first = (first_sweep and ci == 0 and j == 0)
                        last = (last_sweep and ci == len(chunks) - 1
                                and j == qc - 1)
                        nc.tensor.matmul(
                            ps_scores[:],
                            zv[t][:, 127 - ql:255 - ql],
                            S[:, ts(j, Y)],
                            start=first, stop=last)
                    q0 += qc

            def softmax_block(qb, ps_scores, split_out):
                mx = small.tile([128, 1], f32, tag="mx")
                nc.vector.reduce_max(mx[:], ps_scores[:], axis=mybir.AxisListType.X)
                sim_sb = small.tile([128, 1], f32, tag="sim")
                nc.vector.tensor_add(sim_sb[:], mx[:], vb_bc[:])
                ps_simT = psum_proj.tile([1, 128], f32, tag="hq", name=f"psimT{qb}")
                nc.tensor.transpose(ps_simT[:], sim_sb[:], ident_f32[:])
                sim_row = small.tile([1, 128], f32, tag="simrow")
                nc.vector.tensor_copy(sim_row[:], ps_simT[:])
                nc.sync.dma_start(sim_ext.ap()[0:1, ts(qb, 128)], sim_row[:])
                nmx = small.tile([128, 1], f32, tag="nmx")
                nc.vector.tensor_scalar_mul(nmx[:], mx[:], -1.0)
                e_sb = soft_pool.tile([128, Y], f32, tag="e")
                sum_e = small.tile([128, 1], f32, tag="sum")
                nc.scalar.activation(e_sb[:], ps_scores[:], AF.Exp,
                                     bias=nmx[:, 0:1], accum_out=sum_e[:, 0:1])
                rinv = small.tile([128, 1], f32, tag="rinv")
                nc.vector.reciprocal(rinv[:], sum_e[:])
                if split_out:
                    for h in range(2):
                        nc.vector.tensor_scalar_mul(
                            e_sb[:, ts(h, Y // 2)], e_sb[:, ts(h, Y // 2)],
                            rinv[:, 0:1])
                        nc.sync.dma_start(
                            att_ext.ap()[ts(qb, 128), ts(h, Y // 2)],
                            e_sb[:, ts(h, Y // 2)])
                else:
                    nc.vector.tensor_scalar_mul(e_sb[:], e_sb[:], rinv[:, 0:1])
                    nc.sync.dma_start(att_ext.ap()[ts(qb, 128), :], e_sb[:])

            FULL = [QC] * NCH                       # 8 x 16
            HEAD = [4, 4, 8] + [QC] * (NCH - 1)     # ramp up ACT early
            TAIL = [QC] * (NCH - 1) + [8, 4, 4]     # shrink exposed PE tail

            # Emission order drives the schedule: minimal t=0 path first so
            # the ACT main loop starts ASAP; the whole t=1 side hides under
            # the first (qb=0, t=0) tanh sweep (~56us of ACT work).
            lat(wy_ext, AT, wyT, only_i=0)
            lat(y_ext, YTILES, yT)
            proj_hy(0)
            lat(wq_ext, AT, wqT, only_i=0)
            lat(query_ext, QTILES, qT, only_i=0)
            small_consts()
            proj_hq(0, 0)

            scores0 = psum_sc.tile([128, Y], f32, tag="scores", name="scores0")
            sweep(0, 0, scores0, HEAD, first_sweep=True, last_sweep=False)

            # t=1 prologue now: fills engine idle slots under the sweep above
            lat(wy_ext, AT, wyT, only_i=1)
            proj_hy(1)
            lat(wq_ext, AT, wqT, only_i=1)
            lat(query_ext, QTILES, qT, only_i=1)
            proj_hq(1, 0)
            proj_hq(0, 1)
            proj_hq(1, 1)

            sweep(0, 1, scores0, FULL, first_sweep=False, last_sweep=True)
            softmax_block(0, scores0, split_out=False)

            scores1 = psum_sc.tile([128, Y], f32, tag="scores", name="scores1")
            sweep(1, 0, scores1, FULL, first_sweep=True, last_sweep=False)
            sweep(1, 1, scores1, TAIL, first_sweep=False, last_sweep=True)
            softmax_block(1, scores1, split_out=True)

    nc.compile()
    return nc


def _get_nc():
    global _cached
    if _cached is None:
        _cached = _build()
    return _cached


def kernel(query, y, Wq_w, Wq_b, Wy_w, Wy_b, v_w, v_b):
    from concourse.bass_utils import run_bass_kernel_spmd

    nc = _get_nc()
    in_maps = []
    for b in range(B):
        in_maps.append({
            "query": np.ascontiguousarray(query[b], dtype=np.float32),
            "y": np.ascontiguousarray(y[b], dtype=np.float32),
            "Wq_w": np.ascontiguousarray(Wq_w, dtype=np.float32),
            "Wq_b": np.ascontiguousarray(Wq_b, dtype=np.float32),
            "Wy_w": np.ascontiguousarray(Wy_w, dtype=np.float32),
            "Wy_b": np.ascontiguousarray(Wy_b, dtype=np.float32),
            "v_w": np.ascontiguousarray(v_w, dtype=np.float32),
            "v_b": np.ascontiguousarray(v_b, dtype=np.float32),
        })
    res = run_bass_kernel_spmd(nc, in_maps, core_ids=list(range(B)))
    att = np.stack([res.results[b]["att"] for b in range(B)])
    sim = np.stack([res.results[b]["sim"] for b in range(B)])
    return att.astype(np.float32), sim.astype(np.float32)
